# revision 12
# baseline (speedup 1.0000x reference)
"""Trainium2 Bass kernel for an attention block (B=4, C=64, H=W=64).

reference:
    xf = x.reshape(B, C, N)                      # N = H*W = 4096
    qkv = w_qkv @ xf + b_qkv                     # [B, 3C, N]
    q, k, v = split(qkv)
    attn = softmax(q^T k / sqrt(C), axis=-1)     # [B, N, N]
    out = w_proj @ (v @ attn^T) + b_proj + x

Sharding: 8 cores = (batch sample, query half). Each core receives its
sample's tokens ROTATED so its own 2048 queries are always columns
0:2048 (attention is permutation-invariant over keys). Each core
computes K/V for its sample plus the attention output for its queries;
no collectives.

Weight prep folds the q/k projections into A = Wk^T Wq (scores =
x_m . (A x_q + c), c = Wk^T b_q; the k-side bias cancels in softmax)
and the output projection into the v projection (w_vp = w_proj @ w_v;
b_eff = w_proj @ b_v + b_proj since softmax rows sum to one). The
q-side bias c rides as a 65th row of the folded projection against a
65th all-ones row of the bf16 input, so QW comes out of one matmul.

Speed layout: scores run in bf16 (1 cyc/row). The attention weights E
are written in fp8 e5m2 — natively by the scalar engine's Exp and via a
Schraudolph bit-trick (uint8 = 0.7213*s + 60) on the vector engine,
alternating key chunks between the only two engines that can read PSUM
(GPSIMD cannot). V^T is stored in fp8 e4m3, letting the AV contraction
run as DoubleRow fp8 matmuls: each call contracts a PAIR of 128-key
chunks at 0.5 cyc/row (4x fewer PE cycles than bf16 AV). The softmax
denominator is a second DoubleRow matmul against a replicated-ones fp8
stationary (32 identical rows; dual-fp8 ldweights requires >=32 columns
and dst partition 0, so it cannot share the AV call or psum tile).
Queries advance in 512-wide blocks so a score tile is one PSUM bank,
giving a 5-deep ring that hides the exp latency behind the tensor
engine (a 2-deep ring of 1024-wide tiles serializes exp -> scores).
AV pairs are emitted 2 chunks behind their scores for the same reason.
Division by the denominator happens after the folded output projection
(it commutes), broadcast across partitions with a contraction-1 matmul.
"""

import numpy as np

import concourse.bass as bass
import concourse.tile as tile
from concourse import mybir
from concourse.bass_utils import run_bass_kernel_spmd

B, C = 4, 64
CP = C + 1        # channels + ones row for the folded q bias
N = 4096          # H*W tokens
QH = N // 2       # queries per core
QB = 512          # queries per block
NQB = QH // QB    # 4 blocks
MC = 128          # keys per chunk
NMC = N // MC     # 32 chunks
NPAIR = NMC // 2  # 16 DoubleRow pairs

_F32 = mybir.dt.float32
_F32R = mybir.dt.float32r
_BF16 = mybir.dt.bfloat16
_E4 = mybir.dt.float8e4
_E5 = mybir.dt.float8e5
_U8 = mybir.dt.uint8
_I16 = mybir.dt.int16
_EXP = mybir.ActivationFunctionType.Exp
_DR = mybir.MatmulPerfMode.DoubleRow
_ADD = mybir.AluOpType.add
_MULT = mybir.AluOpType.mult

# e5m2 Schraudolph: e5m2_bits(exp(s/8)) ~= uint8(s * (0.125*4*log2 e) + 60)
_SCH_A = 0.125 * 4.0 * 1.4426950408889634
_SCH_B = 60.0

# exp engine per pair tile (global index 0..63): D=vector, A=scalar.
# Extra A at two block starts balances Act 34 / DVE 30 (DVE runs the
# boundary reciprocals/multiplies there).
_EXP_PAT = list("DA" * 32)
_EXP_PAT[16] = "A"
_EXP_PAT[48] = "A"


def _split_excess_waits(nc):
    """walrus accepts at most one sync wait per instruction; move extras
    onto NoOps spliced just before it."""
    for f in nc.m.functions:
        for bb in f.blocks:
            new_insts = []
            changed = False
            for inst in bb.instructions:
                si = inst.sync_info
                if si is not None and si.on_wait and len(si.on_wait) > 1:
                    waits = list(si.on_wait)
                    extra, keep = waits[:-1], waits[-1:]
                    for w in extra:
                        nop = mybir.InstNoOp(name=nc.get_next_instruction_name())
                        nop.engine = inst.engine
                        nop.sync_info = mybir.SyncInfo(on_wait=[w], on_update=[])
                        nc.register_instruction(nop)
                        new_insts.append(nop)
                    si.on_wait = keep
                    changed = True
                new_insts.append(inst)
            if changed:
                bb.instructions = new_insts


def build_graph():
    nc = bass.Bass("TRN2", target_bir_lowering=False, debug=False)

    xb_ext = nc.declare_dram_parameter("xb", [CP, N], _I16, isOutput=False)
    xq_ext = nc.declare_dram_parameter("xq", [C, QH], _F32, isOutput=False)
    wqkc_ext = nc.declare_dram_parameter("w_qkc", [CP, C], _I16, isOutput=False)
    wvpT_ext = nc.declare_dram_parameter("w_vpT", [C, C], _I16, isOutput=False)
    beff_ext = nc.declare_dram_parameter("b_eff", [C, 1], _F32, isOutput=False)
    onesw_ext = nc.declare_dram_parameter("onesw", [MC, C], _U8, isOutput=False)
    ones1_ext = nc.declare_dram_parameter("ones1", [1, C], _F32, isOutput=False)
    out_ext = nc.declare_dram_parameter("out", [C, QH], _F32, isOutput=True)

    with (
        nc.allow_low_precision(reason="fp8 attention weights by design"),
        tile.TileContext(nc) as tc,
        tc.tile_pool(name="consts", bufs=1) as consts,
        # PSUM (8 banks): scores ring 3x[128,1024]=6 (one chunk PAIR per
        # tile), av 1x[64,512]=1, denom 1x[32,512]=1 (epilogue pb shares
        # the dn slot at block boundaries)
        tc.tile_pool(name="spool", bufs=3, space="PSUM") as spool,
        tc.tile_pool(name="avpool", bufs=1, space="PSUM") as avpool,
        tc.tile_pool(name="dnpool", bufs=1, space="PSUM") as dnpool,
        tc.tile_pool(name="ebuf", bufs=4) as ebuf,
        tc.tile_pool(name="obuf", bufs=8) as obuf,
    ):
        XB = consts.tile([CP, N], _BF16, tag="xb")
        XQ = consts.tile([C, QH], _F32, tag="xq")
        WQKC = consts.tile([CP, C], _BF16, tag="wqkc")
        WVP = consts.tile([C, C], _BF16, tag="wvp")
        BEFF = consts.tile([C, 1], _F32, tag="beff")
        OW = consts.tile([MC, C], _E5, tag="ow")
        ONES1 = consts.tile([1, C], _F32R, tag="ones1")
        QW = consts.tile([C, QH], _BF16, tag="qw")
        VT3 = consts.tile([MC, NMC, C], _E4, tag="vt")
        OW2 = OW.rearrange("p (two m) -> p two m", two=2)

        # ---- input DMAs: weights lead, xb key chunks interleave across
        # queues; xq (residual only) comes last ----
        def dma_xb(eng, j):
            eng.dma_start(
                out=XB[:, j * 512 : (j + 1) * 512].bitcast(_I16),
                in_=xb_ext[:, j * 512 : (j + 1) * 512],
            )

        def dma_xq(eng, j):
            eng.dma_start(
                out=XQ[:, j * 512 : (j + 1) * 512],
                in_=xq_ext[:, j * 512 : (j + 1) * 512],
            )

        dma_xb(nc.gpsimd, 0)
        nc.sync.dma_start(out=WQKC.bitcast(_I16), in_=wqkc_ext[:, :])
        dma_xb(nc.sync, 1)
        nc.sync.dma_start(out=WVP.bitcast(_I16), in_=wvpT_ext[:, :])
        dma_xb(nc.gpsimd, 2)
        nc.gpsimd.dma_start(out=OW.bitcast(_U8), in_=onesw_ext[:, :])
        dma_xb(nc.sync, 3)
        dma_xb(nc.gpsimd, 4)
        nc.sync.dma_start(out=ONES1, in_=ones1_ext[:, :].bitcast(_F32R))
        dma_xb(nc.sync, 5)
        dma_xb(nc.gpsimd, 6)
        nc.sync.dma_start(out=BEFF, in_=beff_ext[:, :])
        dma_xb(nc.sync, 7)
        dma_xq(nc.gpsimd, 0)
        dma_xq(nc.sync, 1)
        dma_xq(nc.gpsimd, 2)
        dma_xq(nc.sync, 3)

        # preload the Exp table (1283ns) while DMAs are in flight
        WARM = consts.tile([1, 1], _F32, tag="warm")
        nc.vector.memset(WARM, 0.0)
        nc.scalar.activation(WARM, WARM, _EXP, bias=0.0, scale=1.0)

        # ---- projections (emitted just-in-time around the loop head) ----
        def emit_qw(j):
            # QW chunk j = A x_q + c via the ones-row fold; plain copy out
            lo, hi = j * 512, (j + 1) * 512
            ps = spool.tile([C, 512], _F32, tag="s")
            nc.tensor.matmul(ps, WQKC, XB[:, lo:hi], start=True, stop=True)
            nc.scalar.copy(QW[:, lo:hi], ps)

        def emit_vp(g):
            # projected V^T for key chunks 4g..4g+3, stored fp8 e4m3
            ps = spool.tile([MC, 4, C], _F32, tag="s")
            for i in range(4):
                m = g * 4 + i
                nc.tensor.matmul(
                    ps[:, i, :], XB[0:C, m * MC : (m + 1) * MC], WVP,
                    start=True, stop=True,
                )
            nc.scalar.copy(VT3[:, g * 4 : (g + 1) * 4, :], ps)

        emit_qw(0)
        emit_qw(1)
        emit_vp(0)
        hooks = {}
        hooks.setdefault(1, []).append((emit_qw, 2))
        hooks.setdefault(2, []).append((emit_qw, 3))
        for g in range(1, 8):
            hooks.setdefault(g, []).append((emit_vp, g))

        # ---- attention ----
        def emit_exp(gp, pss, E2):
            edst = E2.rearrange("p two n -> p (two n)")
            if _EXP_PAT[gp] == "A":
                nc.scalar.activation(edst, pss, _EXP, bias=0.0, scale=0.125)
            else:
                nc.vector.tensor_scalar(
                    out=edst.bitcast(_U8), in0=pss,
                    scalar1=_SCH_A, scalar2=_SCH_B, op0=_MULT, op1=_ADD,
                )

        def emit_av(p, E2, pav, pdn):
            # DoubleRow fp8: one call contracts the chunk PAIR (256 keys)
            nc.tensor.matmul(
                pav, VT3[:, 2 * p : 2 * p + 2, :], E2,
                start=(p == 0), stop=(p == NPAIR - 1), perf_mode=_DR,
            )
            nc.tensor.matmul(
                pdn, OW2, E2,
                start=(p == 0), stop=(p == NPAIR - 1), perf_mode=_DR,
            )

        def emit_epilogue(qb, pav, pdn):
            # reciprocal of denominators + drain of the accumulator (frees
            # both psum tiles), then broadcast recip across partitions via
            # contraction-1 matmul (pb shares the dn slot), multiply, add
            # bias + residual, store.
            q0 = qb * QB
            R1 = obuf.tile([1, QB], _F32R, tag="o")
            nc.vector.reciprocal(R1, pdn[0:1, :])
            U = obuf.tile([C, QB], _F32, tag="o")
            nc.scalar.copy(U, pav)
            pb = dnpool.tile([C, QB], _F32, tag="dn", name="pb")
            nc.tensor.matmul(pb, ONES1, R1, start=True, stop=True)
            UN = obuf.tile([C, QB], _F32, tag="o")
            nc.vector.tensor_mul(UN, U, pb)
            O = obuf.tile([C, QB], _F32, tag="o")
            nc.gpsimd.tensor_scalar_add(O, UN, BEFF)
            nc.gpsimd.tensor_add(O, O, XQ[:, q0 : q0 + QB])
            nc.sync.dma_start(out=out_ext[:, q0 : q0 + QB], in_=O)

        avq = []         # (qb, pair, E2) awaiting AV emission (1 pair behind)
        acc = {}         # qb -> (pav, pdn)
        for qb in range(NQB):
            q0 = qb * QB
            for p in range(NPAIR):
                pss = spool.tile([MC, QB * 2], _F32, tag="s")
                for i in (0, 1):
                    nc.tensor.matmul(
                        pss[:, i * QB : (i + 1) * QB],
                        XB[0:C, (2 * p + i) * MC : (2 * p + i + 1) * MC],
                        QW[:, q0 : q0 + QB],
                        start=True, stop=True,
                    )
                if p == 2:
                    pav = avpool.tile([C, QB], _F32, tag="av", name="pav")
                    pdn = dnpool.tile([32, QB], _F32, tag="dn", name="pdn")
                    acc[qb] = (pav, pdn)
                if len(avq) >= 2:
                    pqb, pm, pE2 = avq.pop(0)
                    emit_av(pm, pE2, *acc[pqb])
                    if pm == NPAIR - 1:
                        emit_epilogue(pqb, *acc.pop(pqb))
                E2 = ebuf.tile([MC, 2, QB], _E5, tag="e")
                emit_exp(qb * NPAIR + p, pss, E2)
                avq.append((qb, p, E2))
                if qb == 0:
                    for fn, arg in hooks.get(p, ()):
                        fn(arg)
        # tail: remaining AV pair + epilogue
        while avq:
            pqb, pm, pE2 = avq.pop(0)
            emit_av(pm, pE2, *acc[pqb])
        emit_epilogue(NQB - 1, *acc.pop(NQB - 1))

    _split_excess_waits(nc)
    return nc


_GRAPH_CACHE = {}


def _get_graph():
    if "nc" not in _GRAPH_CACHE:
        _GRAPH_CACHE["nc"] = build_graph()
    return _GRAPH_CACHE["nc"]


def _bf16_bits(a):
    a = np.ascontiguousarray(a.astype(np.float32))
    u = a.view(np.uint32)
    return (((u + 0x7FFF + ((u >> 16) & 1)) >> 16).astype(np.uint16)).view(np.int16)


_ONESW = np.full((MC, C), 0x3C, dtype=np.uint8)  # e5m2 bits of 1.0
_ONES1 = np.ones((1, C), dtype=np.float32)


def make_in_maps(x, w_qkv, b_qkv, w_proj, b_proj):
    xf = np.ascontiguousarray(np.asarray(x, dtype=np.float32).reshape(B, C, N))
    w_qkv = np.asarray(w_qkv, dtype=np.float32)
    b_qkv = np.asarray(b_qkv, dtype=np.float32)
    w_proj = np.asarray(w_proj, dtype=np.float32)
    b_proj = np.asarray(b_proj, dtype=np.float32)

    # scores = x_m . (A x_q + c): A = Wk^T Wq, c = Wk^T b_q; stationary is
    # [A^T; c^T] against x extended with an all-ones row
    A = w_qkv[C : 2 * C].T @ w_qkv[0:C]
    c = w_qkv[C : 2 * C].T @ b_qkv[0:C]
    w_qkc = _bf16_bits(np.concatenate([A.T, c[None, :]], axis=0))
    # fold the output projection into the v projection
    w_vpT = _bf16_bits((w_proj @ w_qkv[2 * C :]).T)
    b_eff = (w_proj @ b_qkv[2 * C :] + b_proj).reshape(C, 1).astype(np.float32)

    in_maps = []
    for core in range(8):
        b, h = divmod(core, 2)
        # rotate tokens so this core's queries are columns 0:QH
        xr = np.ascontiguousarray(np.roll(xf[b], -h * QH, axis=1))
        xb = np.concatenate([xr, np.ones((1, N), dtype=np.float32)], axis=0)
        in_maps.append(
            {
                "xb": _bf16_bits(xb),
                "xq": np.ascontiguousarray(xr[:, :QH]),
                "w_qkc": w_qkc,
                "w_vpT": w_vpT,
                "b_eff": b_eff,
                "onesw": _ONESW,
                "ones1": _ONES1,
            }
        )
    return in_maps


def kernel(x, w_qkv, b_qkv, w_proj, b_proj):
    x = np.asarray(x)
    nc = _get_graph()
    in_maps = make_in_maps(x, w_qkv, b_qkv, w_proj, b_proj)
    res = run_bass_kernel_spmd(nc, in_maps, core_ids=list(range(8)))
    out = np.empty((B, C, N), dtype=np.float32)
    for core in range(8):
        b, h = divmod(core, 2)
        out[b][:, h * QH : (h + 1) * QH] = res.results[core]["out"]
    return out.reshape(x.shape).astype(np.float32)


# revision 16
# speedup vs baseline: 1.0228x; 1.0228x over previous
"""Trainium2 Bass kernel for an attention block (B=4, C=64, H=W=64).

reference:
    xf = x.reshape(B, C, N)                      # N = H*W = 4096
    qkv = w_qkv @ xf + b_qkv                     # [B, 3C, N]
    q, k, v = split(qkv)
    attn = softmax(q^T k / sqrt(C), axis=-1)     # [B, N, N]
    out = w_proj @ (v @ attn^T) + b_proj + x

Sharding: 8 cores = (batch sample, query half). Each core receives its
sample's tokens ROTATED so its own 2048 queries are always columns
0:2048 (attention is permutation-invariant over keys). Each core
computes K/V for its sample plus the attention output for its queries;
no collectives.

Weight prep folds the q/k projections into A = Wk^T Wq (scores =
x_m . (A x_q + c), c = Wk^T b_q; the k-side bias cancels in softmax)
and the output projection into the v projection (w_vp = w_proj @ w_v;
b_eff = w_proj @ b_v + b_proj since softmax rows sum to one). The
q-side bias c rides as a 65th row of the folded projection against a
65th all-ones row of the bf16 input, so QW comes out of one matmul.

Speed layout: scores run in bf16 (1 cyc/row). The attention weights E
are written in fp8 e5m2 — natively by the scalar engine's Exp and via a
Schraudolph bit-trick (uint8 = 0.7213*s + 60) on the vector engine,
alternating key chunks between the only two engines that can read PSUM
(GPSIMD cannot). V^T is stored in fp8 e4m3, letting the AV contraction
run as DoubleRow fp8 matmuls: each call contracts a PAIR of 128-key
chunks at 0.5 cyc/row (4x fewer PE cycles than bf16 AV). The softmax
denominator is a second DoubleRow matmul against a replicated-ones fp8
stationary (32 identical rows; dual-fp8 ldweights requires >=32 columns
and dst partition 0, so it cannot share the AV call or psum tile).
Queries advance in 512-wide blocks so a score tile is one PSUM bank,
giving a 5-deep ring that hides the exp latency behind the tensor
engine (a 2-deep ring of 1024-wide tiles serializes exp -> scores).
AV pairs are emitted 2 chunks behind their scores for the same reason.
Division by the denominator happens after the folded output projection
(it commutes), broadcast across partitions with a contraction-1 matmul.
"""

import numpy as np

import concourse.bass as bass
import concourse.tile as tile
from concourse import mybir
from concourse.bass_utils import run_bass_kernel_spmd

B, C = 4, 64
CP = C + 1        # channels + ones row for the folded q bias
N = 4096          # H*W tokens
QH = N // 2       # queries per core
QB = 512          # queries per block
NQB = QH // QB    # 4 blocks
MC = 128          # keys per chunk
NMC = N // MC     # 32 chunks
NPAIR = NMC // 2  # 16 DoubleRow pairs

_F32 = mybir.dt.float32
_F32R = mybir.dt.float32r
_BF16 = mybir.dt.bfloat16
_E4 = mybir.dt.float8e4
_E5 = mybir.dt.float8e5
_U8 = mybir.dt.uint8
_I16 = mybir.dt.int16
_EXP = mybir.ActivationFunctionType.Exp
_DR = mybir.MatmulPerfMode.DoubleRow
_ADD = mybir.AluOpType.add
_MULT = mybir.AluOpType.mult

# e5m2 Schraudolph: e5m2_bits(exp(s/8)) ~= uint8(s * (0.125*4*log2 e) + 60)
_SCH_A = 0.125 * 4.0 * 1.4426950408889634
_SCH_B = 60.0

# exp engine per pair tile (global index 0..63): D=vector, A=scalar.
# Extra A at two block starts balances the boundary reciprocals and
# multiplies that keep the vector engine busy there.
_EXP_PAT = list("DA" * 32)
_EXP_PAT[16] = "A"
_EXP_PAT[48] = "A"
_EXP_PAT[41] = "D"


def _split_excess_waits(nc):
    """walrus accepts at most one sync wait per instruction; move extras
    onto NoOps spliced just before it."""
    for f in nc.m.functions:
        for bb in f.blocks:
            new_insts = []
            changed = False
            for inst in bb.instructions:
                si = inst.sync_info
                if si is not None and si.on_wait and len(si.on_wait) > 1:
                    waits = list(si.on_wait)
                    extra, keep = waits[:-1], waits[-1:]
                    for w in extra:
                        nop = mybir.InstNoOp(name=nc.get_next_instruction_name())
                        nop.engine = inst.engine
                        nop.sync_info = mybir.SyncInfo(on_wait=[w], on_update=[])
                        nc.register_instruction(nop)
                        new_insts.append(nop)
                    si.on_wait = keep
                    changed = True
                new_insts.append(inst)
            if changed:
                bb.instructions = new_insts


def build_graph():
    nc = bass.Bass("TRN2", target_bir_lowering=False, debug=False)

    xb_ext = nc.declare_dram_parameter("xb", [CP, N], _I16, isOutput=False)
    xq_ext = nc.declare_dram_parameter("xq", [C, QH], _F32, isOutput=False)
    wqkc_ext = nc.declare_dram_parameter("w_qkc", [CP, C], _I16, isOutput=False)
    wvpT_ext = nc.declare_dram_parameter("w_vpT", [C, C], _I16, isOutput=False)
    beff_ext = nc.declare_dram_parameter("b_eff", [C, 1], _F32, isOutput=False)
    onesw_ext = nc.declare_dram_parameter("onesw", [MC, C], _U8, isOutput=False)
    ones1_ext = nc.declare_dram_parameter("ones1", [1, C], _F32, isOutput=False)
    out_ext = nc.declare_dram_parameter("out", [C, QH], _F32, isOutput=True)

    with (
        nc.allow_low_precision(reason="fp8 attention weights by design"),
        tile.TileContext(nc) as tc,
        tc.tile_pool(name="consts", bufs=1) as consts,
        # PSUM (8 banks): scores ring 3x[128,1024]=6 (one chunk PAIR per
        # tile), av 1x[64,512]=1, denom 1x[32,512]=1 (epilogue pb shares
        # the dn slot at block boundaries)
        tc.tile_pool(name="spool", bufs=3, space="PSUM") as spool,
        tc.tile_pool(name="avpool", bufs=1, space="PSUM") as avpool,
        tc.tile_pool(name="dnpool", bufs=1, space="PSUM") as dnpool,
        tc.tile_pool(name="ebuf", bufs=6) as ebuf,
        tc.tile_pool(name="obuf", bufs=8) as obuf,
    ):
        XB = consts.tile([CP, N], _BF16, tag="xb")
        XQ = consts.tile([C, QH], _F32, tag="xq")
        WQKC = consts.tile([CP, C], _BF16, tag="wqkc")
        WVP = consts.tile([C, C], _BF16, tag="wvp")
        BEFF = consts.tile([C, 1], _F32, tag="beff")
        OW = consts.tile([MC, C], _E5, tag="ow")
        ONES1 = consts.tile([1, C], _F32R, tag="ones1")
        QW = consts.tile([C, QH], _BF16, tag="qw")
        VT3 = consts.tile([MC, NMC, C], _E4, tag="vt")
        OW2 = OW.rearrange("p (two m) -> p two m", two=2)

        # ---- input DMAs: weights lead, xb key chunks interleave across
        # queues; xq (residual only) comes last ----
        def dma_xb(eng, j):
            eng.dma_start(
                out=XB[:, j * 512 : (j + 1) * 512].bitcast(_I16),
                in_=xb_ext[:, j * 512 : (j + 1) * 512],
            )

        def dma_xq(eng, j):
            eng.dma_start(
                out=XQ[:, j * 512 : (j + 1) * 512],
                in_=xq_ext[:, j * 512 : (j + 1) * 512],
            )

        dma_xb(nc.gpsimd, 0)
        nc.sync.dma_start(out=WQKC.bitcast(_I16), in_=wqkc_ext[:, :])
        dma_xb(nc.sync, 1)
        nc.sync.dma_start(out=WVP.bitcast(_I16), in_=wvpT_ext[:, :])
        dma_xb(nc.gpsimd, 2)
        nc.gpsimd.dma_start(out=OW.bitcast(_U8), in_=onesw_ext[:, :])
        dma_xb(nc.sync, 3)
        dma_xb(nc.gpsimd, 4)
        nc.sync.dma_start(out=ONES1, in_=ones1_ext[:, :].bitcast(_F32R))
        dma_xb(nc.sync, 5)
        dma_xb(nc.gpsimd, 6)
        nc.sync.dma_start(out=BEFF, in_=beff_ext[:, :])
        dma_xb(nc.sync, 7)
        dma_xq(nc.gpsimd, 0)
        dma_xq(nc.sync, 1)
        dma_xq(nc.gpsimd, 2)
        dma_xq(nc.sync, 3)

        # preload the Exp table (1283ns) while DMAs are in flight
        WARM = consts.tile([1, 1], _F32, tag="warm")
        nc.vector.memset(WARM, 0.0)
        nc.scalar.activation(WARM, WARM, _EXP, bias=0.0, scale=1.0)

        # ---- projections (emitted just-in-time around the loop head) ----
        def emit_qw(j):
            # QW chunk j = A x_q + c via the ones-row fold; plain copy out
            lo, hi = j * 512, (j + 1) * 512
            ps = spool.tile([C, 512], _F32, tag="s")
            nc.tensor.matmul(ps, WQKC, XB[:, lo:hi], start=True, stop=True)
            nc.scalar.copy(QW[:, lo:hi], ps)

        def emit_vp(g):
            # projected V^T for key chunks 4g..4g+3, stored fp8 e4m3
            ps = spool.tile([MC, 4, C], _F32, tag="s")
            for i in range(4):
                m = g * 4 + i
                nc.tensor.matmul(
                    ps[:, i, :], XB[0:C, m * MC : (m + 1) * MC], WVP,
                    start=True, stop=True,
                )
            if g >= 5:
                nc.vector.tensor_copy(VT3[:, g * 4 : (g + 1) * 4, :], ps)
            else:
                nc.scalar.copy(VT3[:, g * 4 : (g + 1) * 4, :], ps)

        emit_qw(0)
        emit_qw(1)
        emit_vp(0)
        hooks = {}
        hooks.setdefault(1, []).append((emit_qw, 2))
        hooks.setdefault(2, []).append((emit_qw, 3))
        for g in range(1, 8):
            hooks.setdefault(g, []).append((emit_vp, g))

        # ---- attention ----
        def emit_exp(gp, pss, E2):
            edst = E2.rearrange("p two n -> p (two n)")
            if _EXP_PAT[gp] == "A":
                nc.scalar.activation(edst, pss, _EXP, bias=0.0, scale=0.125)
            else:
                nc.vector.tensor_scalar(
                    out=edst.bitcast(_U8), in0=pss,
                    scalar1=_SCH_A, scalar2=_SCH_B, op0=_MULT, op1=_ADD,
                )

        def emit_av(p, E2, pav, pdn):
            # DoubleRow fp8: one call contracts the chunk PAIR (256 keys)
            nc.tensor.matmul(
                pav, VT3[:, 2 * p : 2 * p + 2, :], E2,
                start=(p == 0), stop=(p == NPAIR - 1), perf_mode=_DR,
            )
            nc.tensor.matmul(
                pdn, OW2, E2,
                start=(p == 0), stop=(p == NPAIR - 1), perf_mode=_DR,
            )

        def epilogue_drain(qb, pav, pdn):
            # stage 1: reciprocal of denominators + drain of the
            # accumulator; frees both psum tiles
            R1 = obuf.tile([1, QB], _F32R, tag="o")
            nc.vector.reciprocal(R1, pdn[0:1, :])
            U = obuf.tile([C, QB], _F32, tag="o")
            nc.scalar.copy(U, pav)
            return (qb, R1, U)

        def epilogue_store(qb, R1, U):
            # stage 2 (a pair later, so the PE never waits on the recip):
            # broadcast recip across partitions via contraction-1 matmul
            # (pb shares the dn slot), multiply, add bias + residual, store.
            q0 = qb * QB
            pb = dnpool.tile([C, QB], _F32, tag="dn", name="pb")
            nc.tensor.matmul(pb, ONES1, R1, start=True, stop=True)
            UN = obuf.tile([C, QB], _F32, tag="o")
            nc.vector.tensor_mul(UN, U, pb)
            O = obuf.tile([C, QB], _F32, tag="o")
            nc.gpsimd.tensor_scalar_add(O, UN, BEFF)
            nc.gpsimd.tensor_add(O, O, XQ[:, q0 : q0 + QB])
            nc.sync.dma_start(out=out_ext[:, q0 : q0 + QB], in_=O)

        avq = []         # (qb, pair, E2) awaiting AV emission (3 pairs behind)
        acc = {}         # qb -> (pav, pdn)
        drained = None   # (qb, R1, U) between epilogue stages
        for qb in range(NQB):
            for p in range(NPAIR):
                pss = spool.tile([MC, QB * 2], _F32, tag="s")
                for i in (0, 1):
                    nc.tensor.matmul(
                        pss[:, i * QB : (i + 1) * QB],
                        XB[0:C, (2 * p + i) * MC : (2 * p + i + 1) * MC],
                        QW[:, qb * QB : (qb + 1) * QB],
                        start=True, stop=True,
                    )
                if drained is not None:
                    epilogue_store(*drained)
                    drained = None
                if p == 3:
                    pav = avpool.tile([C, QB], _F32, tag="av", name="pav")
                    pdn = dnpool.tile([32, QB], _F32, tag="dn", name="pdn")
                    acc[qb] = (pav, pdn)
                if len(avq) >= 3:
                    pqb, pm, pE2 = avq.pop(0)
                    emit_av(pm, pE2, *acc[pqb])
                    if pm == NPAIR - 1:
                        drained = epilogue_drain(pqb, *acc.pop(pqb))
                E2 = ebuf.tile([MC, 2, QB], _E5, tag="e")
                emit_exp(qb * NPAIR + p, pss, E2)
                avq.append((qb, p, E2))
                if qb == 0:
                    for fn, arg in hooks.get(p, ()):
                        fn(arg)
        # tail: remaining AV pairs + epilogue
        while avq:
            pqb, pm, pE2 = avq.pop(0)
            emit_av(pm, pE2, *acc[pqb])
        drained = epilogue_drain(NQB - 1, *acc.pop(NQB - 1))
        epilogue_store(*drained)

    _split_excess_waits(nc)
    return nc


_GRAPH_CACHE = {}


def _get_graph():
    if "nc" not in _GRAPH_CACHE:
        _GRAPH_CACHE["nc"] = build_graph()
    return _GRAPH_CACHE["nc"]


def _bf16_bits(a):
    a = np.ascontiguousarray(a.astype(np.float32))
    u = a.view(np.uint32)
    return (((u + 0x7FFF + ((u >> 16) & 1)) >> 16).astype(np.uint16)).view(np.int16)


_ONESW = np.full((MC, C), 0x3C, dtype=np.uint8)  # e5m2 bits of 1.0
_ONES1 = np.ones((1, C), dtype=np.float32)


def make_in_maps(x, w_qkv, b_qkv, w_proj, b_proj):
    xf = np.ascontiguousarray(np.asarray(x, dtype=np.float32).reshape(B, C, N))
    w_qkv = np.asarray(w_qkv, dtype=np.float32)
    b_qkv = np.asarray(b_qkv, dtype=np.float32)
    w_proj = np.asarray(w_proj, dtype=np.float32)
    b_proj = np.asarray(b_proj, dtype=np.float32)

    # scores = x_m . (A x_q + c): A = Wk^T Wq, c = Wk^T b_q; stationary is
    # [A^T; c^T] against x extended with an all-ones row
    A = w_qkv[C : 2 * C].T @ w_qkv[0:C]
    c = w_qkv[C : 2 * C].T @ b_qkv[0:C]
    w_qkc = _bf16_bits(np.concatenate([A.T, c[None, :]], axis=0))
    # fold the output projection into the v projection
    w_vpT = _bf16_bits((w_proj @ w_qkv[2 * C :]).T)
    b_eff = (w_proj @ b_qkv[2 * C :] + b_proj).reshape(C, 1).astype(np.float32)

    in_maps = []
    for core in range(8):
        b, h = divmod(core, 2)
        # rotate tokens so this core's queries are columns 0:QH
        xr = np.ascontiguousarray(np.roll(xf[b], -h * QH, axis=1))
        xb = np.concatenate([xr, np.ones((1, N), dtype=np.float32)], axis=0)
        in_maps.append(
            {
                "xb": _bf16_bits(xb),
                "xq": np.ascontiguousarray(xr[:, :QH]),
                "w_qkc": w_qkc,
                "w_vpT": w_vpT,
                "b_eff": b_eff,
                "onesw": _ONESW,
                "ones1": _ONES1,
            }
        )
    return in_maps


def kernel(x, w_qkv, b_qkv, w_proj, b_proj):
    x = np.asarray(x)
    nc = _get_graph()
    in_maps = make_in_maps(x, w_qkv, b_qkv, w_proj, b_proj)
    res = run_bass_kernel_spmd(nc, in_maps, core_ids=list(range(8)))
    out = np.empty((B, C, N), dtype=np.float32)
    for core in range(8):
        b, h = divmod(core, 2)
        out[b][:, h * QH : (h + 1) * QH] = res.results[core]["out"]
    return out.reshape(x.shape).astype(np.float32)
